# revision 38
# baseline (speedup 1.0000x reference)
"""Trainium2 Bass kernel for nn_CWALayerv3 (avgpool8 -> dw-conv resblock ->
instance-norm -> channel-gram attention -> masked mean).

Sharding: 8 cores = (batch b in 0..3) x (channel half in 0..1). Each core owns
128 channels of one batch image (channel-per-partition layout).

The whole device pipeline runs in bf16 (the 2e-2 rel-err gate prices this in;
measured ~6e-3): x is cast to bf16 on the host during sharding, which halves
the dominant HBM stream. With all 8 cores streaming, per-core HBM bandwidth is
~290 GB/s (716 GB/s per stack shared by an NC pair, minus interference), so
phase 1 is paced by the 67 MB x read per core.

Phase 1 (per core): stream x [128c, H, W] bf16 in 16-row chunks (two pool
windows each) on the SP HWDGE ring; the 8x8 sum-pool is a bf16 tensor_tensor
add-tree (row pairs 4+4 -> 2+2 -> 1+1 across both windows per op, each
leveraging the DVE 2x_1p packed mode, unlike a single multi-axis
tensor_reduce which runs 1x) plus one innermost-8 reduce; depthwise
3x3 convs as 9 scalar_tensor_tensor MACs each (DVE-only opcode; per-partition
f32 weight scalars, residual folded into conv2's first tap), pipelined in row
blocks so conv work, PE transposes, and per-block zT stores all overlap the
stream. Outputs: zT [S, 128] bf16 (spatial-major) + per-channel sum/sumsq
accumulated in f32 on the scalar engine.

Host: instance-norm scalars in float64 from the sums (z = 64*z_true is
corrected: a = gamma/sqrt(var_z + 64^2*eps), b = beta - mu_z*a, so
f = a*z + b is the exact normalized feature).

Phase 2 (per core): zTb [S, C] bf16 (own half's columns first) loaded in
grouped chunks; accumulating bf16 matmuls -> Zgram [128, C] in PSUM;
reconstruct the f-gram G = a_c a_d Zg + u_c b_d + b_c w_d with
host-precomputed outer products; sigmoid; mask-multiply (mask pre-scaled by
1/C, diagonal zeroed) + row-sum -> out [128].
"""

import contextlib

import ml_dtypes
import numpy as np

import concourse.bass as bass
import concourse.bacc as bacc
import concourse.mybir as mybir
import concourse.tile as tile
from concourse.bass_utils import run_bass_kernel_spmd

F32 = mybir.dt.float32
BF16 = mybir.dt.bfloat16
Alu = mybir.AluOpType
Act = mybir.ActivationFunctionType

EPS = 1e-5
CH = 128          # channels per core
POOL = 8          # avg-pool window
BLK = 8           # conv pipeline block = BLK pooled rows
BACKEND = "hw"    # "hw" | "sim"

_program_cache = {}


def _blocks(PH, PW):
    # conv pipeline blocks (r0, blk): big 16-row blocks mid-stream (halves
    # per-op DVE init overhead), split the last 16 pooled rows into 4-row
    # blocks to shorten the end-of-stream serial chain
    if PH >= 32 and (PH - 16) % 16 == 0:
        return [(i * 16, 16) for i in range((PH - 16) // 16)] + [
            (PH - 16 + 4 * j, 4) for j in range(4)
        ]
    NB = PH // BLK
    assert NB * BLK == PH
    if NB >= 3:
        return [(i * BLK, BLK) for i in range(NB - 2)] + [
            (PH - 16 + 4 * j, 4) for j in range(4)
        ]
    return [(i * BLK, BLK) for i in range(NB)]


def build_phase1(H, W, debug=False, reps=1, chunk_bufs=5, gp_split=0):
    """One core's phase-1 program.

    x [CH, H, W] bf16 (host-cast during sharding; the 2e-2 gate prices this
    in) -> zT [S, CH] bf16 (unnormalized, 64x-scaled pooled activations,
    spatial-major via PE transposes), zs/zzs [CH, NZB] f32 (per-block sum and
    sum-of-squares). gp_split is retained for API compatibility (gpsimd
    offload measured slower on HW). reps>1 wraps the body in an on-device
    For_i.
    """
    PH, PW = H // POOL, W // POOL
    S = PH * PW
    NCHUNK = H // (2 * POOL)
    assert NCHUNK * 2 * POOL == H
    blocks = _blocks(PH, PW)
    NZB = len(blocks)

    nc = bacc.Bacc("TRN2", target_bir_lowering=False, debug=debug)
    x_d = nc.dram_tensor("x", [CH, H, W], BF16, kind="ExternalInput")
    w1_d = nc.dram_tensor("w1t", [CH, 9], F32, kind="ExternalInput")
    w2_d = nc.dram_tensor("w2t", [CH, 9], F32, kind="ExternalInput")
    id_d = nc.dram_tensor("ident", [128, 128], BF16, kind="ExternalInput")
    zt_d = nc.dram_tensor("zT", [S, CH], BF16, kind="ExternalOutput")
    zs_d = nc.dram_tensor("zs", [CH, NZB], F32, kind="ExternalOutput")
    zzs_d = nc.dram_tensor("zzs", [CH, NZB], F32, kind="ExternalOutput")

    with tile.TileContext(nc) as tc, nc.allow_low_precision(
        reason="bf16 pooled activations are within the 2e-2 gate; "
        "norm stats accumulate in f32 via accum_out"
    ):
        with tc.tile_pool(name="consts", bufs=1) as consts, (
            tc.tile_pool(name="imgs", bufs=1)) as imgs, (
            tc.tile_pool(name="stats", bufs=1)) as stats:
            w1t = consts.tile([CH, 9], F32)
            w2t = consts.tile([CH, 9], F32)
            idt = consts.tile([128, 128], BF16)
            nc.scalar.dma_start(w1t[:], w1_d.ap())
            nc.scalar.dma_start(w2t[:], w2_d.ap())
            nc.scalar.dma_start(idt[:], id_d.ap())

            # padded pooled image P and conv1 output Q: [CH, PH+2, PW+2].
            # Border zeroing is hoisted out of the timing loop: the interior
            # is fully rewritten every iteration, the borders never dirtied.
            P = imgs.tile([CH, PH + 2, PW + 2], BF16)
            Q = imgs.tile([CH, PH + 2, PW + 2], BF16)
            sums_z = stats.tile([CH, NZB], F32)
            sums_zz = stats.tile([CH, NZB], F32)
            nc.gpsimd.memset(P[:], 0.0)
            nc.gpsimd.memset(Q[:], 0.0)

            loop = tc.For_i(0, reps, 1) if reps > 1 else contextlib.nullcontext()
            with loop, (
                tc.tile_pool(name="chunks", bufs=chunk_bufs)) as chunks, (
                tc.tile_pool(name="tree", bufs=4)) as tree, (
                tc.tile_pool(name="accs", bufs=3)) as accs, (
                tc.tile_pool(name="zbs", bufs=2)) as zbs, (
                tc.tile_pool(name="stages", bufs=2)) as stages, (
                tc.tile_pool(name="psum", bufs=4, space=bass.MemorySpace.PSUM)) as psp:

                def conv1_block(i):
                    r0, blk = blocks[i]
                    acc = accs.tile([CH, blk, PW], BF16, name="acc", tag="acc")
                    nc.vector.tensor_scalar(
                        acc[:], P[:, r0 : r0 + blk, 0:PW],
                        w1t[:, 0:1], None, Alu.mult,
                    )
                    for t in range(1, 9):
                        ky, kx = divmod(t, 3)
                        nxt = accs.tile([CH, blk, PW], BF16, name="acc", tag="acc")
                        nc.vector.scalar_tensor_tensor(
                            nxt[:],
                            P[:, r0 + ky : r0 + ky + blk, kx : kx + PW],
                            w1t[:, t : t + 1], acc[:],
                            op0=Alu.mult, op1=Alu.add,
                        )
                        acc = nxt
                    nc.scalar.activation(
                        Q[:, r0 + 1 : r0 + 1 + blk, 1 : 1 + PW], acc[:], Act.Relu
                    )

                def conv2_block(i):
                    r0, blk = blocks[i]
                    acc = accs.tile([CH, blk, PW], BF16, name="acc", tag="acc")
                    nc.vector.scalar_tensor_tensor(
                        acc[:], Q[:, r0 : r0 + blk, 0:PW], w2t[:, 0:1],
                        P[:, r0 + 1 : r0 + 1 + blk, 1 : 1 + PW],
                        op0=Alu.mult, op1=Alu.add,
                    )
                    for t in range(1, 9):
                        ky, kx = divmod(t, 3)
                        nxt = accs.tile([CH, blk, PW], BF16, name="acc", tag="acc")
                        nc.vector.scalar_tensor_tensor(
                            nxt[:],
                            Q[:, r0 + ky : r0 + ky + blk, kx : kx + PW],
                            w2t[:, t : t + 1], acc[:],
                            op0=Alu.mult, op1=Alu.add,
                        )
                        acc = nxt
                    zb = zbs.tile([CH, blk, PW], BF16, name="zb", tag="zb")
                    nc.scalar.activation(
                        zb[:], acc[:], Act.Relu, accum_out=sums_z[:, i : i + 1]
                    )
                    sq = accs.tile([CH, blk, PW], BF16, name="sq", tag="acc")
                    nc.scalar.activation(
                        sq[:], zb[:], Act.Square,
                        accum_out=sums_zz[:, i : i + 1],
                    )
                    # transpose block to spatial-major and store
                    tpb = blk * PW // 128
                    zf = zb[:].rearrange("p a b -> p (a b)")
                    stage = stages.tile([CH, tpb, 128], BF16, name="stage",
                                        tag="stage")
                    for j in range(tpb):
                        tp = psp.tile([128, 128], BF16, name="tp", tag="tp")
                        nc.tensor.transpose(
                            tp[:], zf[:, 128 * j : 128 * (j + 1)], idt[:]
                        )
                        nc.scalar.copy(stage[:, j, :], tp[:])
                    nc.scalar.dma_start(
                        zt_d.ap()[r0 * PW : (r0 + blk) * PW, :].rearrange(
                            "(j s) c -> s j c", s=128
                        ),
                        stage[:],
                    )

                def pool_chunk(k):
                    # 16-row chunk = 2 pool windows; one 4-level bf16
                    # add-tree handles both windows per op (each op runs the
                    # DVE 2x_1p packed mode), then one innermost-8 reduce
                    # yields two P rows. Half the DMA count of 8-row chunks:
                    # longer HBM bursts + half the per-op init overhead.
                    ch = chunks.tile([CH, 2 * POOL, W], BF16, name="ch",
                                     tag="ch")
                    nc.sync.dma_start(
                        ch[:], x_d.ap()[:, 2 * POOL * k : 2 * POOL * (k + 1), :]
                    )
                    v = ch[:].rearrange(
                        "p (win half r) w -> p win half r w", win=2, half=2)
                    t4 = tree.tile([CH, 2, 4, W], BF16, name="t4", tag="t4")
                    nc.vector.tensor_tensor(
                        t4[:], v[:, :, 0], v[:, :, 1], Alu.add)
                    t2 = tree.tile([CH, 2, 2, W], BF16, name="t2", tag="t2")
                    nc.vector.tensor_tensor(
                        t2[:], t4[:, :, 0:2, :], t4[:, :, 2:4, :], Alu.add)
                    t1 = tree.tile([CH, 2, W], BF16, name="t1", tag="t1")
                    nc.vector.tensor_tensor(
                        t1[:], t2[:, :, 0, :], t2[:, :, 1, :], Alu.add)
                    # innermost-8 sum as packed pair-adds (2x_1p) instead of
                    # a tensor_reduce, which has no fast mode (1x)
                    tv = t1[:].rearrange("p r (wp wi) -> p r wp wi", wi=POOL)
                    u4 = tree.tile([CH, 2, PW, 4], BF16, name="u4", tag="u4")
                    nc.vector.tensor_tensor(
                        u4[:], tv[:, :, :, 0:4], tv[:, :, :, 4:8], Alu.add)
                    u2 = tree.tile([CH, 2, PW, 2], BF16, name="u2", tag="u2")
                    nc.vector.tensor_tensor(
                        u2[:], u4[:, :, :, 0:2], u4[:, :, :, 2:4], Alu.add)
                    nc.vector.tensor_tensor(
                        P[:, 2 * k + 1 : 2 * k + 3, 1 : 1 + PW],
                        u2[:, :, :, 0], u2[:, :, :, 1], Alu.add)

                # stream + fused 8x8 sum-pool; conv blocks interleave
                c1_done = c2_done = 0
                for k in range(NCHUNK):
                    pool_chunk(k)
                    while (
                        c1_done < NZB - 1
                        and 2 * k + 1 >= blocks[c1_done][0] + blocks[c1_done][1]
                    ):
                        conv1_block(c1_done)
                        c1_done += 1
                        while c2_done < c1_done - 1:
                            conv2_block(c2_done)
                            c2_done += 1
                conv1_block(NZB - 1)
                for i in range(c2_done, NZB):
                    conv2_block(i)

                nc.scalar.dma_start(zs_d.ap(), sums_z[:])
                nc.scalar.dma_start(zzs_d.ap(), sums_zz[:])

    nc.compile()
    return nc


def build_phase2(S, C, debug=False, reps=1, grp=8):
    """One core's phase-2 program.

    zTb [S, C] bf16 (spatial-major, all channels of this batch, own half's
    columns first), correction matrices, mask [CH, C] -> out [CH, 1].
    """
    NT = S // 128
    nc = bacc.Bacc("TRN2", target_bir_lowering=False, debug=debug)
    ztb_d = nc.dram_tensor("zTb", [S, C], BF16, kind="ExternalInput")
    mask_d = nc.dram_tensor("mask", [CH, C], F32, kind="ExternalInput")
    aout_d = nc.dram_tensor("Aout", [CH, C], F32, kind="ExternalInput")
    mcorr_d = nc.dram_tensor("Mcorr", [CH, C], F32, kind="ExternalInput")
    out_d = nc.dram_tensor("out", [CH, 1], F32, kind="ExternalOutput")
    ztb_v = ztb_d.ap().rearrange("(j s) c -> j s c", s=128)

    with tile.TileContext(nc) as tc:
        with tc.tile_pool(name="consts", bufs=1) as consts:
            mask = consts.tile([CH, C], F32)
            aout = consts.tile([CH, C], F32)
            mcorr = consts.tile([CH, C], F32)
            nc.scalar.dma_start(mask[:], mask_d.ap())
            nc.scalar.dma_start(aout[:], aout_d.ap())
            nc.scalar.dma_start(mcorr[:], mcorr_d.ap())

            loop = tc.For_i(0, reps, 1) if reps > 1 else contextlib.nullcontext()
            with loop, (
                tc.tile_pool(name="sb", bufs=1)) as sb, (
                tc.tile_pool(name="psum", bufs=1, space=bass.MemorySpace.PSUM)) as psp:
                ztb = sb.tile([128, NT, C], BF16)
                G = psp.tile([CH, C], F32)
                for g0 in range(0, NT, grp):
                    g1 = min(g0 + grp, NT)
                    nc.sync.dma_start(
                        ztb[:, g0:g1, :],
                        ztb_v[g0:g1].rearrange("j s c -> s j c"),
                    )
                    for j in range(g0, g1):
                        nc.tensor.matmul(
                            G[:], ztb[:, j, 0:CH], ztb[:, j, :],
                            start=(j == 0), stop=(j == NT - 1),
                        )
                # G_f = Zg o Aout + Mcorr  (host-precomputed outer products)
                g1t = sb.tile([CH, C], F32)
                g2t = sb.tile([CH, C], F32)
                nc.vector.tensor_tensor(g1t[:], G[:], aout[:], Alu.mult)
                nc.vector.tensor_tensor(g2t[:], g1t[:], mcorr[:], Alu.add)
                sig = sb.tile([CH, C], F32)
                nc.scalar.activation(sig[:], g2t[:], Act.Sigmoid)
                scratch = sb.tile([CH, C], F32)
                res = sb.tile([CH, 1], F32)
                nc.vector.tensor_tensor(scratch[:], sig[:], mask[:], Alu.mult)
                nc.vector.tensor_reduce(
                    res[:], scratch[:], axis=mybir.AxisListType.X, op=Alu.add
                )
                nc.scalar.dma_start(out_d.ap(), res[:])

    nc.compile()
    return nc


def _get_program(key, builder):
    if key not in _program_cache:
        _program_cache[key] = builder()
    return _program_cache[key]


def _run(nc, in_maps):
    if BACKEND == "sim":
        from concourse.bass_interp import CoreSim

        results = []
        for im in in_maps:
            sim = CoreSim(nc, trace=False)
            for name, arr in im.items():
                sim.tensor(name)[:] = arr
            sim.simulate(check_with_hw=False)
            out = {}
            for alloc in nc.m.functions[0].allocations:
                if (
                    isinstance(alloc, mybir.MemoryLocationSet)
                    and alloc.kind == "ExternalOutput"
                ):
                    name = alloc.memorylocations[0].name
                    out[name] = np.array(sim.tensor(name))
            results.append(out)
            del sim
        return results
    res = run_bass_kernel_spmd(nc, in_maps, list(range(len(in_maps))))
    return res.results


def kernel(x, w1, w2, gamma, beta):
    x = np.asarray(x)
    w1 = np.asarray(w1)
    w2 = np.asarray(w2)
    gamma = np.asarray(gamma, dtype=np.float64)
    beta = np.asarray(beta, dtype=np.float64)
    B, C, H, W = x.shape
    n_half = C // CH
    assert n_half * CH == C
    PH, PW = H // POOL, W // POOL
    S = PH * PW

    debug = BACKEND == "sim"
    nc1 = _get_program(("p1", H, W, debug), lambda: build_phase1(H, W, debug))
    nc2 = _get_program(("p2", S, C, debug), lambda: build_phase2(S, C, debug))

    w1r = np.ascontiguousarray(w1.reshape(C, 9))
    w2r = np.ascontiguousarray(w2.reshape(C, 9))
    ident = np.eye(128, dtype=ml_dtypes.bfloat16)

    in_maps1 = []
    for b in range(B):
        for h in range(n_half):
            sl = slice(h * CH, (h + 1) * CH)
            in_maps1.append({
                "x": np.ascontiguousarray(x[b, sl]).astype(ml_dtypes.bfloat16),
                "w1t": np.ascontiguousarray(w1r[sl]),
                "w2t": np.ascontiguousarray(w2r[sl]),
                "ident": ident,
            })
    res1 = _run(nc1, in_maps1)

    # gather zc per batch; instance-norm scalars in float64 on host.
    # z = 64*z_true  =>  a = gamma/sqrt(var_z + 64^2*eps), b = beta - mu_z*a
    # G_f = a_c a_d Zg + u_c b_d + b_c w_d with u = a*Sz, w = a*Sz + S*b
    ztb, corr = [], []
    for b in range(B):
        parts = [res1[b * n_half + h] for h in range(n_half)]
        ztb.append(np.concatenate([p["zT"] for p in parts], axis=1))
        Sz = np.concatenate(
            [p["zs"].astype(np.float64).sum(1) for p in parts]
        )
        Szz = np.concatenate(
            [p["zzs"].astype(np.float64).sum(1) for p in parts]
        )
        mu = Sz / S
        var = Szz / S - mu * mu
        a = gamma / np.sqrt(var + float(POOL**4) * EPS)
        bb = beta - mu * a
        u = a * Sz
        w = u + S * bb
        corr.append((a, bb, u, w))

    mask_full = (1.0 - np.eye(C, dtype=np.float32)) / C
    in_maps2 = []
    for b in range(B):
        a, bb, u, w = corr[b]
        for h in range(n_half):
            sl = slice(h * CH, (h + 1) * CH)
            # column order: own half first (row-sum is perm-invariant)
            perm = np.r_[np.arange(h * CH, (h + 1) * CH),
                         np.arange(0, h * CH),
                         np.arange((h + 1) * CH, C)]
            in_maps2.append({
                "zTb": np.ascontiguousarray(ztb[b][:, perm]),
                "mask": np.ascontiguousarray(mask_full[sl][:, perm]),
                "Aout": np.outer(a[sl], a[perm]).astype(np.float32),
                "Mcorr": (np.outer(u[sl], bb[perm])
                          + np.outer(bb[sl], w[perm])).astype(np.float32),
            })
    res2 = _run(nc2, in_maps2)

    out = np.empty((B, C), dtype=np.float32)
    for b in range(B):
        for h in range(n_half):
            out[b, h * CH : (h + 1) * CH] = res2[b * n_half + h]["out"][:, 0]
    return out
